# revision 16
# baseline (speedup 1.0000x reference)
"""GatedLSTM Trainium2 kernel: data-parallel over batch (8 cores x batch-8).

Per core: x-phase precomputes mlx[b,t,:] = sigmoid(x Wm^T)*(x W^T) for gates
(c,f,o) into DRAM scratch; scan phase runs the 256-step recurrence with
h^T as the matmul stationary operand and U^T streamed through 4 concurrent
PE column-tiles. The unused input gate `i` of the module is skipped.
"""

import os
import numpy as np
import concourse.mybir as mybir
import concourse.tile as tile
from concourse import bacc
from concourse.bass_utils import run_bass_kernel_spmd

FP16 = mybir.dt.float16
FP32 = mybir.dt.float32
SIG = mybir.ActivationFunctionType.Sigmoid
TANH = mybir.ActivationFunctionType.Tanh
MULT = mybir.AluOpType.mult
ADD = mybir.AluOpType.add

LAST_RESULT = None
B, T, IN, H = 64, 256, 512, 1024
NCORES = 8
BL = B // NCORES          # 8 batch rows per core
ROWS = BL * T             # 2048 (b*T + t)
NU = 6 * H                # 6144 matmul cols (3 gates x pre/lin)
NX = 3 * H                # 3072 mlx cols (3 gates)
T_STEPS = int(os.environ.get("GL_T_STEPS", T))


def build(t_steps=T):
    nc = bacc.Bacc("TRN2", target_bir_lowering=False, debug=False)

    xT_d = nc.dram_tensor("xT", [4, 128, ROWS], FP16, kind="ExternalInput")
    w_d = nc.dram_tensor("w", [4, 128, NU], FP16, kind="ExternalInput")
    u_d = nc.dram_tensor("u", [8, 128, NU], FP16, kind="ExternalInput")
    bias_d = nc.dram_tensor("bias", [1, NX], FP16, kind="ExternalInput")
    iaug_d = nc.dram_tensor("iaug", [9, 8], FP16, kind="ExternalInput")
    sel_d = nc.dram_tensor("sel", [104, 32], FP16, kind="ExternalInput")
    ys_d = nc.dram_tensor("ys", [BL, T, H], FP16, kind="ExternalOutput")
    cT_d = nc.dram_tensor("cT", [104, 256], FP32, kind="ExternalOutput")
    mlx_d = nc.dram_tensor("mlx_scratch", [BL, T, NX], FP16, kind="Internal")

    with tile.TileContext(nc) as tc:
        with tc.tile_pool(name="res", bufs=1) as res:
            u_sb = res.tile([128, 8, NU], FP16)
            iaug = res.tile([9, 8], FP16)
            sel = res.tile([104, 32], FP16)
            nc.sync.dma_start(iaug[:], iaug_d[:])
            nc.sync.dma_start(sel[:], sel_d[:])

            # ---------------- x-phase ----------------
            with (
                tc.tile_pool(name="xsb", bufs=1) as xsb,
                tc.tile_pool(name="xew", bufs=8) as xew,
                tc.tile_pool(name="xps", bufs=1, space="PSUM") as xps,
            ):
                w_sb = xsb.tile([128, 4, NU], FP16)
                xT_sb = xsb.tile([128, 4, ROWS], FP16)
                for k in range(4):
                    nc.sync.dma_start(w_sb[:, k, :], w_d[k])
                    nc.sync.dma_start(xT_sb[:, k, :], xT_d[k])
                # U is only needed by the scan; queue its (12.6 MB) load after
                # the x-phase operands so the first x matmuls aren't starved.
                for k in range(8):
                    nc.sync.dma_start(u_sb[:, k, :], u_d[k])
                for half in range(2):
                    for n in range(12):          # n = 3q + g
                        q, g = divmod(n, 3)
                        n0 = 1536 * q + 512 * g
                        for r8 in range(8):
                            R = 8 * half + r8
                            px = xps.tile([128, 512], FP32, tag=f"px{r8}")
                            for k in range(4):
                                nc.tensor.matmul(
                                    px[:],
                                    xT_sb[:, k, 128 * R : 128 * R + 128],
                                    w_sb[:, k, n0 : n0 + 512],
                                    start=(k == 0), stop=(k == 3),
                                )
                            spx = xew.tile([128, 256], FP32, tag="spx")
                            nc.scalar.activation(spx[:], px[:, 0:256], SIG)
                            mxs = xew.tile([128, 256], FP16, tag="mxs")
                            nc.vector.tensor_mul(mxs[:], spx[:], px[:, 256:512])
                            b0 = R // 2
                            t0 = (R % 2) * 128
                            nc.sync.dma_start(
                                mlx_d[b0, t0 : t0 + 128,
                                      768 * q + 256 * g : 768 * q + 256 * g + 256],
                                mxs[:],
                            )

            # ---------------- scan phase ----------------
            with (
                tc.tile_pool(name="ssb", bufs=2) as ssb,
                tc.tile_pool(name="smlx", bufs=3) as smlx,
                tc.tile_pool(name="spg", bufs=1, space="PSUM") as spg,
                tc.tile_pool(name="smx", bufs=2, space="PSUM") as smx,
                tc.tile_pool(name="spt", bufs=1, space="PSUM") as spt,
            ):
                def load_mlx(t):
                    m = smlx.tile([9, NX], FP16, tag="mlx")
                    nc.sync.dma_start(m[0:8, :], mlx_d[:, t, :])
                    nc.sync.dma_start(m[8:9, :], bias_d[:])
                    return m

                def mx_pass(mt):
                    mx = smx.tile([128, 768], FP32, tag="mx")
                    for q in range(4):
                        for c0, cw in ((0, 512), (512, 256)):
                            nc.tensor.matmul(
                                mx[32 * q : 32 * q + 8, c0 : c0 + cw],
                                iaug[:],
                                mt[:, 768 * q + c0 : 768 * q + c0 + cw],
                                start=True, stop=True,
                                tile_position=(0, 32 * q),
                            )
                    return mx

                def transpose_h(h):
                    pT = spt.tile([128, 2, 32], FP32, tag="pT")
                    for X in range(2):
                        for cb in range(4):
                            nc.tensor.matmul(
                                pT[32 * cb : 32 * cb + 32, X, :],
                                h[:, 128 * X + 32 * cb : 128 * X + 32 * cb + 32],
                                sel[:],
                                start=True, stop=True,
                                tile_position=(0, 32 * cb),
                            )
                    hT = ssb.tile([128, 2, 32], FP16, tag="hT")
                    nc.scalar.copy(hT[:], pT[:])
                    return hT

                def store_h(h, t):
                    for q in range(4):
                        nc.sync.dma_start(
                            ys_d[:, t, 256 * q : 256 * q + 256],
                            h[32 * q : 32 * q + 8, :],
                        )

                KORDER = [0, 2, 4, 6, 1, 3, 5, 7]
                mlx_t = load_mlx(0)
                mlx_next = load_mlx(1) if t_steps > 1 else None

                # t = 0: h=0 -> gates from mx alone
                mx = mx_pass(mlx_t)
                gf = ssb.tile([104, 256], FP32, tag="gf")
                go = ssb.tile([104, 256], FP32, tag="go")
                c_t = ssb.tile([104, 256], FP32, tag="c")
                nc.scalar.activation(c_t[:], mx[0:104, 0:256], TANH)
                nc.scalar.activation(gf[:], mx[0:104, 256:512], SIG)
                nc.scalar.activation(go[:], mx[0:104, 512:768], SIG)
                h_t = ssb.tile([104, 256], FP16, tag="h")
                nc.vector.tensor_mul(h_t[:], go[:], c_t[:])
                store_h(h_t, 0)
                hT = transpose_h(h_t)

                for t in range(1, t_steps):
                    mlx_t = mlx_next
                    mlx_next = load_mlx(t + 1) if t + 1 < t_steps else None
                    mx = mx_pass(mlx_t)
                    pg = [
                        spg.tile([128, 512], FP32, tag=f"pg{r}", name=f"pg{r}")
                        for r in range(3)
                    ]
                    # round c: full 512-col k-loop
                    for i, k in enumerate(KORDER):
                        lhs = hT[:, k % 2, 8 * (k // 2) : 8 * (k // 2) + 8]
                        for q in range(4):
                            nc.tensor.matmul(
                                pg[0][32 * q : 32 * q + 8, :],
                                lhs,
                                u_sb[:, k, 1536 * q : 1536 * q + 512],
                                start=(i == 0), stop=(i == 7),
                                tile_position=(0, 32 * q),
                            )
                    # rounds f, o: two half-k-loops (cols 256h..256h+256 per group)
                    for r in (1, 2):
                        for hf in range(2):
                            for i, k in enumerate(KORDER):
                                lhs = hT[:, k % 2, 8 * (k // 2) : 8 * (k // 2) + 8]
                                for q in range(4):
                                    n0 = 1536 * q + 512 * r + 256 * hf
                                    nc.tensor.matmul(
                                        pg[r][32 * q : 32 * q + 8, 256 * hf : 256 * hf + 256],
                                        lhs,
                                        u_sb[:, k, n0 : n0 + 256],
                                        start=(i == 0), stop=(i == 7),
                                        tile_position=(0, 32 * q),
                                    )
                    # z gate (round c) full width
                    sp = ssb.tile([104, 256], FP32, tag="sp")
                    nc.scalar.activation(sp[:], pg[0][0:104, 0:256], SIG)
                    mlh = ssb.tile([104, 256], FP32, tag="mlh")
                    nc.vector.tensor_mul(mlh[:], sp[:], pg[0][0:104, 256:512])
                    gin = ssb.tile([104, 256], FP32, tag="gin")
                    nc.vector.tensor_add(gin[:], mlh[:], mx[0:104, 0:256])
                    z_t = ssb.tile([104, 256], FP32, tag="gz")
                    nc.scalar.activation(z_t[:], gin[:], TANH)
                    # f gate + c update, per half
                    c_prev = c_t
                    c_t = ssb.tile([104, 256], FP32, tag="c")
                    for hf in range(2):
                        cs = slice(128 * hf, 128 * hf + 128)
                        spf = ssb.tile([104, 128], FP32, tag="spf")
                        nc.scalar.activation(
                            spf[:], pg[1][0:104, 256 * hf : 256 * hf + 128], SIG
                        )
                        mlhf = ssb.tile([104, 128], FP32, tag="mlhf")
                        nc.vector.tensor_mul(
                            mlhf[:], spf[:], pg[1][0:104, 256 * hf + 128 : 256 * hf + 256]
                        )
                        ginf = ssb.tile([104, 128], FP32, tag="ginf")
                        nc.vector.tensor_add(
                            ginf[:], mlhf[:], mx[0:104, 256 + 128 * hf : 384 + 128 * hf]
                        )
                        f_h = ssb.tile([104, 128], FP32, tag="gfh")
                        nc.scalar.activation(f_h[:], ginf[:], SIG)
                        fc = ssb.tile([104, 128], FP32, tag="fc")
                        nc.vector.tensor_mul(fc[:], f_h[:], c_prev[:, cs])
                        nc.vector.tensor_add(c_t[:, cs], fc[:], z_t[:, cs])
                    h_t = ssb.tile([104, 256], FP16, tag="h")
                    last = t + 1 >= t_steps
                    pT = None if last else spt.tile([128, 2, 32], FP32, tag="pT")
                    hT_new = None if last else ssb.tile([128, 2, 32], FP16, tag="hT")
                    for hf in range(2):
                        spo = ssb.tile([104, 128], FP32, tag="spo")
                        nc.scalar.activation(
                            spo[:], pg[2][0:104, 256 * hf : 256 * hf + 128], SIG
                        )
                        mlho = ssb.tile([104, 128], FP32, tag="mlho")
                        nc.vector.tensor_mul(
                            mlho[:], spo[:], pg[2][0:104, 256 * hf + 128 : 256 * hf + 256]
                        )
                        gino = ssb.tile([104, 128], FP32, tag="gino")
                        nc.vector.tensor_add(
                            gino[:], mlho[:], mx[0:104, 512 + 128 * hf : 640 + 128 * hf]
                        )
                        go_h = ssb.tile([104, 128], FP32, tag="goh")
                        nc.scalar.activation(go_h[:], gino[:], SIG)
                        nc.vector.tensor_mul(
                            h_t[:, 128 * hf : 128 * hf + 128],
                            go_h[:],
                            c_t[:, 128 * hf : 128 * hf + 128],
                        )
                        if not last:
                            for cb in range(4):
                                nc.tensor.matmul(
                                    pT[32 * cb : 32 * cb + 32, hf, :],
                                    h_t[:, 128 * hf + 32 * cb : 128 * hf + 32 * cb + 32],
                                    sel[:],
                                    start=True, stop=True,
                                    tile_position=(0, 32 * cb),
                                )
                            nc.scalar.copy(hT_new[:, hf, :], pT[:, hf, :])
                    store_h(h_t, t)
                    hT = hT_new
                nc.sync.dma_start(cT_d[:], c_t[:])
    nc.compile()
    return nc


def _prep_weights(kw):
    """Build host-side constant tensors (layout/cast only)."""
    gorder = ("c", "f", "o")
    w_all = np.zeros((512, NU), np.float32)
    u_all = np.zeros((1024, NU), np.float32)
    bias = np.zeros((NX,), np.float32)
    for g_i, g in enumerate(gorder):
        Wl, Wm = kw[f"W_{g}"], kw[f"W_{g}_mask"]
        Ul, Um = kw[f"U_{g}"], kw[f"U_{g}_mask"]
        for q in range(4):
            n0 = 1536 * q + 512 * g_i
            w_all[:, n0 : n0 + 256] = Wm[256 * q : 256 * q + 256, :].T
            w_all[:, n0 + 256 : n0 + 512] = Wl[256 * q : 256 * q + 256, :].T
            if g_i < 1:
                u_all[:, n0 : n0 + 256] = Um[256 * q : 256 * q + 256, :].T
                u_all[:, n0 + 256 : n0 + 512] = Ul[256 * q : 256 * q + 256, :].T
            else:
                # f/o gates: [pre_h0 lin_h0 pre_h1 lin_h1] interleave (128 each)
                for hh in range(2):
                    base = n0 + 256 * hh
                    r0 = 256 * q + 128 * hh
                    u_all[:, base : base + 128] = Um[r0 : r0 + 128, :].T
                    u_all[:, base + 128 : base + 256] = Ul[r0 : r0 + 128, :].T
            bias[768 * q + 256 * g_i : 768 * q + 256 * g_i + 256] = kw[f"b_{g}"][
                256 * q : 256 * q + 256
            ]
    w4 = w_all.reshape(4, 128, NU).astype(np.float16)
    u8 = u_all.reshape(8, 128, NU).astype(np.float16)
    bias16 = bias.reshape(1, NX).astype(np.float16)
    iaug = np.zeros((9, 8), np.float16)
    iaug[0:8, 0:8] = np.eye(8)
    iaug[8, :] = 1.0
    sel = np.zeros((104, 32), np.float16)
    for q in range(4):
        for b in range(8):
            sel[32 * q + b, 8 * q + b] = 1.0
    return w4, u8, bias16, iaug, sel


def kernel(**inputs):
    x = np.asarray(inputs["input"], np.float32)
    w4, u8, bias16, iaug, sel = _prep_weights(inputs)
    nc = build(T_STEPS)
    in_maps = []
    for c in range(NCORES):
        xc = x[BL * c : BL * c + BL].reshape(ROWS, IN)
        xT = np.ascontiguousarray(xc.T.reshape(4, 128, ROWS)).astype(np.float16)
        in_maps.append(
            dict(xT=xT, w=w4, u=u8, bias=bias16, iaug=iaug, sel=sel)
        )
    res = run_bass_kernel_spmd(nc, in_maps, core_ids=list(range(NCORES)))
    global LAST_RESULT
    LAST_RESULT = res
    out = np.zeros((B, T, H), np.float32)
    cT = np.zeros((B, H), np.float32)
    for c in range(NCORES):
        ys = res.results[c]["ys"].astype(np.float32)
        out[BL * c : BL * c + BL] = ys
        cc = res.results[c]["cT"]
        for q in range(4):
            cT[BL * c : BL * c + BL, 256 * q : 256 * q + 256] = cc[
                32 * q : 32 * q + 8, :
            ]
    hT = out[:, T_STEPS - 1, :].copy()
    return out, (hT[None], cT[None])


# revision 17
# speedup vs baseline: 1.0000x; 1.0000x over previous
"""GatedLSTM Trainium2 kernel: data-parallel over batch (8 cores x batch-8).

Per core: x-phase precomputes mlx[b,t,:] = sigmoid(x Wm^T)*(x W^T) for gates
(c,f,o) into DRAM scratch; scan phase runs the 256-step recurrence with
h^T as the matmul stationary operand and U^T streamed through 4 concurrent
PE column-tiles. The unused input gate `i` of the module is skipped.
"""

import os
import numpy as np
import concourse.mybir as mybir
import concourse.tile as tile
from concourse import bacc
from concourse.bass_utils import run_bass_kernel_spmd

FP16 = mybir.dt.float16
FP32 = mybir.dt.float32
SIG = mybir.ActivationFunctionType.Sigmoid
TANH = mybir.ActivationFunctionType.Tanh
MULT = mybir.AluOpType.mult
ADD = mybir.AluOpType.add

LAST_RESULT = None
B, T, IN, H = 64, 256, 512, 1024
NCORES = 8
BL = B // NCORES          # 8 batch rows per core
ROWS = BL * T             # 2048 (b*T + t)
NU = 6 * H                # 6144 matmul cols (3 gates x pre/lin)
NX = 3 * H                # 3072 mlx cols (3 gates)
T_STEPS = int(os.environ.get("GL_T_STEPS", T))


def build(t_steps=T):
    nc = bacc.Bacc("TRN2", target_bir_lowering=False, debug=False)

    xT_d = nc.dram_tensor("xT", [4, 128, ROWS], FP16, kind="ExternalInput")
    w_d = nc.dram_tensor("w", [4, 128, NU], FP16, kind="ExternalInput")
    u_d = nc.dram_tensor("u", [8, 128, NU], FP16, kind="ExternalInput")
    bias_d = nc.dram_tensor("bias", [1, NX], FP16, kind="ExternalInput")
    iaug_d = nc.dram_tensor("iaug", [9, 8], FP16, kind="ExternalInput")
    sel_d = nc.dram_tensor("sel", [104, 32], FP16, kind="ExternalInput")
    ys_d = nc.dram_tensor("ys", [BL, T, H], FP16, kind="ExternalOutput")
    cT_d = nc.dram_tensor("cT", [104, 256], FP32, kind="ExternalOutput")
    mlx_d = nc.dram_tensor("mlx_scratch", [BL, T, NX], FP16, kind="Internal")

    with tile.TileContext(nc) as tc:
        with tc.tile_pool(name="res", bufs=1) as res:
            u_sb = res.tile([128, 8, NU], FP16)
            iaug = res.tile([9, 8], FP16)
            sel = res.tile([104, 32], FP16)
            nc.sync.dma_start(iaug[:], iaug_d[:])
            nc.sync.dma_start(sel[:], sel_d[:])

            # ---------------- x-phase ----------------
            with (
                tc.tile_pool(name="xsb", bufs=1) as xsb,
                tc.tile_pool(name="xew", bufs=8) as xew,
                tc.tile_pool(name="xps", bufs=1, space="PSUM") as xps,
            ):
                w_sb = xsb.tile([128, 4, NU], FP16)
                xT_sb = xsb.tile([128, 4, ROWS], FP16)
                for k in range(4):
                    nc.sync.dma_start(xT_sb[:, k, :], xT_d[k])
                # quarter-major so the first n-chunks' weights land first
                for q in range(4):
                    for k in range(4):
                        nc.sync.dma_start(
                            w_sb[:, k, 1536 * q : 1536 * q + 1536],
                            w_d[k, :, 1536 * q : 1536 * q + 1536],
                        )
                # U is only needed by the scan; queue its (12.6 MB) load after
                # the x-phase operands so the first x matmuls aren't starved.
                for k in range(8):
                    nc.sync.dma_start(u_sb[:, k, :], u_d[k])
                for half in range(2):
                    for n in range(12):          # n = 3q + g
                        q, g = divmod(n, 3)
                        n0 = 1536 * q + 512 * g
                        for r8 in range(8):
                            R = 8 * half + r8
                            px = xps.tile([128, 512], FP32, tag=f"px{r8}")
                            for k in range(4):
                                nc.tensor.matmul(
                                    px[:],
                                    xT_sb[:, k, 128 * R : 128 * R + 128],
                                    w_sb[:, k, n0 : n0 + 512],
                                    start=(k == 0), stop=(k == 3),
                                )
                            spx = xew.tile([128, 256], FP32, tag="spx")
                            nc.scalar.activation(spx[:], px[:, 0:256], SIG)
                            mxs = xew.tile([128, 256], FP16, tag="mxs")
                            nc.vector.tensor_mul(mxs[:], spx[:], px[:, 256:512])
                            b0 = R // 2
                            t0 = (R % 2) * 128
                            nc.sync.dma_start(
                                mlx_d[b0, t0 : t0 + 128,
                                      768 * q + 256 * g : 768 * q + 256 * g + 256],
                                mxs[:],
                            )

            # ---------------- scan phase ----------------
            with (
                tc.tile_pool(name="ssb", bufs=2) as ssb,
                tc.tile_pool(name="smlx", bufs=3) as smlx,
                tc.tile_pool(name="spg", bufs=1, space="PSUM") as spg,
                tc.tile_pool(name="smx", bufs=2, space="PSUM") as smx,
                tc.tile_pool(name="spt", bufs=1, space="PSUM") as spt,
            ):
                def load_mlx(t):
                    m = smlx.tile([9, NX], FP16, tag="mlx")
                    nc.sync.dma_start(m[0:8, :], mlx_d[:, t, :])
                    nc.sync.dma_start(m[8:9, :], bias_d[:])
                    return m

                def mx_pass(mt):
                    mx = smx.tile([128, 768], FP32, tag="mx")
                    for q in range(4):
                        for c0, cw in ((0, 512), (512, 256)):
                            nc.tensor.matmul(
                                mx[32 * q : 32 * q + 8, c0 : c0 + cw],
                                iaug[:],
                                mt[:, 768 * q + c0 : 768 * q + c0 + cw],
                                start=True, stop=True,
                                tile_position=(0, 32 * q),
                            )
                    return mx

                def transpose_h(h):
                    pT = spt.tile([128, 2, 32], FP32, tag="pT")
                    for X in range(2):
                        for cb in range(4):
                            nc.tensor.matmul(
                                pT[32 * cb : 32 * cb + 32, X, :],
                                h[:, 128 * X + 32 * cb : 128 * X + 32 * cb + 32],
                                sel[:],
                                start=True, stop=True,
                                tile_position=(0, 32 * cb),
                            )
                    hT = ssb.tile([128, 2, 32], FP16, tag="hT")
                    nc.scalar.copy(hT[:], pT[:])
                    return hT

                def store_h(h, t):
                    for q in range(4):
                        nc.sync.dma_start(
                            ys_d[:, t, 256 * q : 256 * q + 256],
                            h[32 * q : 32 * q + 8, :],
                        )

                KORDER = [0, 2, 4, 6, 1, 3, 5, 7]
                mlx_t = load_mlx(0)
                mlx_next = load_mlx(1) if t_steps > 1 else None

                # t = 0: h=0 -> gates from mx alone
                mx = mx_pass(mlx_t)
                gf = ssb.tile([104, 256], FP32, tag="gf")
                go = ssb.tile([104, 256], FP32, tag="go")
                c_t = ssb.tile([104, 256], FP32, tag="c")
                nc.scalar.activation(c_t[:], mx[0:104, 0:256], TANH)
                nc.scalar.activation(gf[:], mx[0:104, 256:512], SIG)
                nc.scalar.activation(go[:], mx[0:104, 512:768], SIG)
                h_t = ssb.tile([104, 256], FP16, tag="h")
                nc.vector.tensor_mul(h_t[:], go[:], c_t[:])
                store_h(h_t, 0)
                hT = transpose_h(h_t)

                for t in range(1, t_steps):
                    mlx_t = mlx_next
                    mlx_next = load_mlx(t + 1) if t + 1 < t_steps else None
                    mx = mx_pass(mlx_t)
                    pg = [
                        spg.tile([128, 512], FP32, tag=f"pg{r}", name=f"pg{r}")
                        for r in range(3)
                    ]
                    # round c: full 512-col k-loop
                    for i, k in enumerate(KORDER):
                        lhs = hT[:, k % 2, 8 * (k // 2) : 8 * (k // 2) + 8]
                        for q in range(4):
                            nc.tensor.matmul(
                                pg[0][32 * q : 32 * q + 8, :],
                                lhs,
                                u_sb[:, k, 1536 * q : 1536 * q + 512],
                                start=(i == 0), stop=(i == 7),
                                tile_position=(0, 32 * q),
                            )
                    # rounds f, o: two half-k-loops (cols 256h..256h+256 per group)
                    for r in (1, 2):
                        for hf in range(2):
                            for i, k in enumerate(KORDER):
                                lhs = hT[:, k % 2, 8 * (k // 2) : 8 * (k // 2) + 8]
                                for q in range(4):
                                    n0 = 1536 * q + 512 * r + 256 * hf
                                    nc.tensor.matmul(
                                        pg[r][32 * q : 32 * q + 8, 256 * hf : 256 * hf + 256],
                                        lhs,
                                        u_sb[:, k, n0 : n0 + 256],
                                        start=(i == 0), stop=(i == 7),
                                        tile_position=(0, 32 * q),
                                    )
                    # z gate (round c) full width
                    sp = ssb.tile([104, 256], FP32, tag="sp")
                    nc.scalar.activation(sp[:], pg[0][0:104, 0:256], SIG)
                    mlh = ssb.tile([104, 256], FP32, tag="mlh")
                    nc.vector.tensor_mul(mlh[:], sp[:], pg[0][0:104, 256:512])
                    gin = ssb.tile([104, 256], FP32, tag="gin")
                    nc.vector.tensor_add(gin[:], mlh[:], mx[0:104, 0:256])
                    z_t = ssb.tile([104, 256], FP32, tag="gz")
                    nc.scalar.activation(z_t[:], gin[:], TANH)
                    # f gate + c update, per half
                    c_prev = c_t
                    c_t = ssb.tile([104, 256], FP32, tag="c")
                    for hf in range(2):
                        cs = slice(128 * hf, 128 * hf + 128)
                        spf = ssb.tile([104, 128], FP32, tag="spf")
                        nc.scalar.activation(
                            spf[:], pg[1][0:104, 256 * hf : 256 * hf + 128], SIG
                        )
                        mlhf = ssb.tile([104, 128], FP32, tag="mlhf")
                        nc.vector.tensor_mul(
                            mlhf[:], spf[:], pg[1][0:104, 256 * hf + 128 : 256 * hf + 256]
                        )
                        ginf = ssb.tile([104, 128], FP32, tag="ginf")
                        nc.vector.tensor_add(
                            ginf[:], mlhf[:], mx[0:104, 256 + 128 * hf : 384 + 128 * hf]
                        )
                        f_h = ssb.tile([104, 128], FP32, tag="gfh")
                        nc.scalar.activation(f_h[:], ginf[:], SIG)
                        fc = ssb.tile([104, 128], FP32, tag="fc")
                        nc.vector.tensor_mul(fc[:], f_h[:], c_prev[:, cs])
                        nc.vector.tensor_add(c_t[:, cs], fc[:], z_t[:, cs])
                    h_t = ssb.tile([104, 256], FP16, tag="h")
                    last = t + 1 >= t_steps
                    pT = None if last else spt.tile([128, 2, 32], FP32, tag="pT")
                    hT_new = None if last else ssb.tile([128, 2, 32], FP16, tag="hT")
                    for hf in range(2):
                        spo = ssb.tile([104, 128], FP32, tag="spo")
                        nc.scalar.activation(
                            spo[:], pg[2][0:104, 256 * hf : 256 * hf + 128], SIG
                        )
                        mlho = ssb.tile([104, 128], FP32, tag="mlho")
                        nc.vector.tensor_mul(
                            mlho[:], spo[:], pg[2][0:104, 256 * hf + 128 : 256 * hf + 256]
                        )
                        gino = ssb.tile([104, 128], FP32, tag="gino")
                        nc.vector.tensor_add(
                            gino[:], mlho[:], mx[0:104, 512 + 128 * hf : 640 + 128 * hf]
                        )
                        go_h = ssb.tile([104, 128], FP32, tag="goh")
                        nc.scalar.activation(go_h[:], gino[:], SIG)
                        nc.vector.tensor_mul(
                            h_t[:, 128 * hf : 128 * hf + 128],
                            go_h[:],
                            c_t[:, 128 * hf : 128 * hf + 128],
                        )
                        if not last:
                            for cb in range(4):
                                nc.tensor.matmul(
                                    pT[32 * cb : 32 * cb + 32, hf, :],
                                    h_t[:, 128 * hf + 32 * cb : 128 * hf + 32 * cb + 32],
                                    sel[:],
                                    start=True, stop=True,
                                    tile_position=(0, 32 * cb),
                                )
                            nc.scalar.copy(hT_new[:, hf, :], pT[:, hf, :])
                    store_h(h_t, t)
                    hT = hT_new
                nc.sync.dma_start(cT_d[:], c_t[:])
    nc.compile()
    return nc


def _prep_weights(kw):
    """Build host-side constant tensors (layout/cast only)."""
    gorder = ("c", "f", "o")
    w_all = np.zeros((512, NU), np.float32)
    u_all = np.zeros((1024, NU), np.float32)
    bias = np.zeros((NX,), np.float32)
    for g_i, g in enumerate(gorder):
        Wl, Wm = kw[f"W_{g}"], kw[f"W_{g}_mask"]
        Ul, Um = kw[f"U_{g}"], kw[f"U_{g}_mask"]
        for q in range(4):
            n0 = 1536 * q + 512 * g_i
            w_all[:, n0 : n0 + 256] = Wm[256 * q : 256 * q + 256, :].T
            w_all[:, n0 + 256 : n0 + 512] = Wl[256 * q : 256 * q + 256, :].T
            if g_i < 1:
                u_all[:, n0 : n0 + 256] = Um[256 * q : 256 * q + 256, :].T
                u_all[:, n0 + 256 : n0 + 512] = Ul[256 * q : 256 * q + 256, :].T
            else:
                # f/o gates: [pre_h0 lin_h0 pre_h1 lin_h1] interleave (128 each)
                for hh in range(2):
                    base = n0 + 256 * hh
                    r0 = 256 * q + 128 * hh
                    u_all[:, base : base + 128] = Um[r0 : r0 + 128, :].T
                    u_all[:, base + 128 : base + 256] = Ul[r0 : r0 + 128, :].T
            bias[768 * q + 256 * g_i : 768 * q + 256 * g_i + 256] = kw[f"b_{g}"][
                256 * q : 256 * q + 256
            ]
    w4 = w_all.reshape(4, 128, NU).astype(np.float16)
    u8 = u_all.reshape(8, 128, NU).astype(np.float16)
    bias16 = bias.reshape(1, NX).astype(np.float16)
    iaug = np.zeros((9, 8), np.float16)
    iaug[0:8, 0:8] = np.eye(8)
    iaug[8, :] = 1.0
    sel = np.zeros((104, 32), np.float16)
    for q in range(4):
        for b in range(8):
            sel[32 * q + b, 8 * q + b] = 1.0
    return w4, u8, bias16, iaug, sel


def kernel(**inputs):
    x = np.asarray(inputs["input"], np.float32)
    w4, u8, bias16, iaug, sel = _prep_weights(inputs)
    nc = build(T_STEPS)
    in_maps = []
    for c in range(NCORES):
        xc = x[BL * c : BL * c + BL].reshape(ROWS, IN)
        xT = np.ascontiguousarray(xc.T.reshape(4, 128, ROWS)).astype(np.float16)
        in_maps.append(
            dict(xT=xT, w=w4, u=u8, bias=bias16, iaug=iaug, sel=sel)
        )
    res = run_bass_kernel_spmd(nc, in_maps, core_ids=list(range(NCORES)))
    global LAST_RESULT
    LAST_RESULT = res
    out = np.zeros((B, T, H), np.float32)
    cT = np.zeros((B, H), np.float32)
    for c in range(NCORES):
        ys = res.results[c]["ys"].astype(np.float32)
        out[BL * c : BL * c + BL] = ys
        cc = res.results[c]["cT"]
        for q in range(4):
            cT[BL * c : BL * c + BL, 256 * q : 256 * q + 256] = cc[
                32 * q : 32 * q + 8, :
            ]
    hT = out[:, T_STEPS - 1, :].copy()
    return out, (hT[None], cT[None])


# revision 22
# speedup vs baseline: 1.0866x; 1.0865x over previous
"""GatedLSTM Trainium2 kernel: data-parallel over batch (8 cores x batch-8).

Per core: x-phase precomputes mlx[b,t,:] = sigmoid(x Wm^T)*(x W^T) for gates
(c,f,o) into DRAM scratch; scan phase runs the 256-step recurrence with
h^T as the matmul stationary operand and U^T streamed through 4 concurrent
PE column-tiles. The unused input gate `i` of the module is skipped.
"""

import os
import numpy as np
import concourse.mybir as mybir
import concourse.tile as tile
from concourse import bacc
from concourse.bass_utils import run_bass_kernel_spmd

FP16 = mybir.dt.float16
FP32 = mybir.dt.float32
SIG = mybir.ActivationFunctionType.Sigmoid
TANH = mybir.ActivationFunctionType.Tanh
MULT = mybir.AluOpType.mult
ADD = mybir.AluOpType.add

LAST_RESULT = None
B, T, IN, H = 64, 256, 512, 1024
NCORES = 8
BL = B // NCORES          # 8 batch rows per core
ROWS = BL * T             # 2048 (b*T + t)
NU = 6 * H                # 6144 matmul cols (3 gates x pre/lin)
NX = 3 * H                # 3072 mlx cols (3 gates)
T_STEPS = int(os.environ.get("GL_T_STEPS", T))


def build(t_steps=T):
    nc = bacc.Bacc("TRN2", target_bir_lowering=False, debug=False)

    xT_d = nc.dram_tensor("xT", [4, 128, ROWS], FP16, kind="ExternalInput")
    w_d = nc.dram_tensor("w", [4, 128, NU], FP16, kind="ExternalInput")
    u_d = nc.dram_tensor("u", [8, 128, NU], FP16, kind="ExternalInput")
    bias_d = nc.dram_tensor("bias", [1, NX], FP16, kind="ExternalInput")
    iaug_d = nc.dram_tensor("iaug", [9, 8], FP16, kind="ExternalInput")
    sel_d = nc.dram_tensor("sel", [104, 32], FP16, kind="ExternalInput")
    ys_d = nc.dram_tensor("ys", [BL, T, H], FP16, kind="ExternalOutput")
    cT_d = nc.dram_tensor("cT", [104, 256], FP32, kind="ExternalOutput")
    mlx_d = nc.dram_tensor("mlx_scratch", [BL, T, NX], FP16, kind="Internal")

    with tile.TileContext(nc) as tc:
        with tc.tile_pool(name="res", bufs=1) as res:
            u_sb = res.tile([128, 8, NU], FP16)
            iaug = res.tile([9, 8], FP16)
            sel = res.tile([104, 32], FP16)
            nc.sync.dma_start(iaug[:], iaug_d[:])
            nc.sync.dma_start(sel[:], sel_d[:])

            # ---------------- x-phase ----------------
            with (
                tc.tile_pool(name="xsb", bufs=1) as xsb,
                tc.tile_pool(name="xew", bufs=8) as xew,
                tc.tile_pool(name="xps", bufs=1, space="PSUM") as xps,
            ):
                w_sb = xsb.tile([128, 4, NU], FP16)
                xT_sb = xsb.tile([128, 4, ROWS], FP16)
                for k in range(4):
                    nc.sync.dma_start(xT_sb[:, k, :], xT_d[k])
                # quarter-major so the first n-chunks' weights land first
                for q in range(4):
                    for k in range(4):
                        nc.sync.dma_start(
                            w_sb[:, k, 1536 * q : 1536 * q + 1536],
                            w_d[k, :, 1536 * q : 1536 * q + 1536],
                        )
                # U is only needed by the scan; queue its (12.6 MB) load after
                # the x-phase operands so the first x matmuls aren't starved.
                for k in range(8):
                    nc.sync.dma_start(u_sb[:, k, :], u_d[k])
                for half in range(2):
                    for n in range(12):          # n = 3q + g
                        q, g = divmod(n, 3)
                        n0 = 1536 * q + 512 * g
                        for r8 in range(8):
                            R = 8 * half + r8
                            px = xps.tile([128, 512], FP32, tag=f"px{r8}")
                            for k in range(4):
                                nc.tensor.matmul(
                                    px[:],
                                    xT_sb[:, k, 128 * R : 128 * R + 128],
                                    w_sb[:, k, n0 : n0 + 512],
                                    start=(k == 0), stop=(k == 3),
                                )
                            spx = xew.tile([128, 256], FP32, tag="spx")
                            nc.scalar.activation(spx[:], px[:, 0:256], SIG)
                            mxs = xew.tile([128, 256], FP16, tag="mxs")
                            nc.vector.tensor_mul(mxs[:], spx[:], px[:, 256:512])
                            b0 = R // 2
                            t0 = (R % 2) * 128
                            nc.sync.dma_start(
                                mlx_d[b0, t0 : t0 + 128,
                                      768 * q + 256 * g : 768 * q + 256 * g + 256],
                                mxs[:],
                            )

            # ---------------- scan phase ----------------
            with (
                tc.tile_pool(name="ssb", bufs=2) as ssb,
                tc.tile_pool(name="smlx", bufs=3) as smlx,
                tc.tile_pool(name="spg", bufs=1, space="PSUM") as spg,
                tc.tile_pool(name="smx", bufs=1, space="PSUM") as smx,
                tc.tile_pool(name="spt", bufs=1, space="PSUM") as spt,
            ):
                def load_mlx(t):
                    m = smlx.tile([9, NX], FP16, tag="mlx")
                    nc.sync.dma_start(m[0:8, :], mlx_d[:, t, :])
                    nc.sync.dma_start(m[8:9, :], bias_d[:])
                    return m

                def mx_pass(mt):
                    mx = smx.tile([128, 768], FP32, tag="mx")
                    for q in range(4):
                        for c0, cw in ((0, 512), (512, 256)):
                            nc.tensor.matmul(
                                mx[32 * q : 32 * q + 8, c0 : c0 + cw],
                                iaug[:],
                                mt[:, 768 * q + c0 : 768 * q + c0 + cw],
                                start=True, stop=True,
                                tile_position=(0, 32 * q),
                            )
                    return mx

                def transpose_h(h):
                    pT = spt.tile([128, 2, 32], FP32, tag="pT")
                    for X in range(2):
                        for cb in range(4):
                            nc.tensor.matmul(
                                pT[32 * cb : 32 * cb + 32, X, :],
                                h[:, 128 * X + 32 * cb : 128 * X + 32 * cb + 32],
                                sel[:],
                                start=True, stop=True,
                                tile_position=(0, 32 * cb),
                            )
                    hT = ssb.tile([128, 2, 32], FP16, tag="hT")
                    nc.scalar.copy(hT[:], pT[:])
                    return hT

                def store_h(h, t):
                    for q in range(4):
                        nc.sync.dma_start(
                            ys_d[:, t, 256 * q : 256 * q + 256],
                            h[32 * q : 32 * q + 8, :],
                        )

                KORDER = [0, 2, 4, 6, 1, 3, 5, 7]
                mlx_t = load_mlx(0)
                mlx_next = load_mlx(1) if t_steps > 1 else None

                # t = 0: h=0 -> gates from mx alone
                mx = mx_pass(mlx_t)
                gf = ssb.tile([104, 256], FP32, tag="gf")
                go = ssb.tile([104, 256], FP32, tag="go")
                c_t = ssb.tile([104, 256], FP32, tag="c")
                nc.scalar.activation(c_t[:], mx[0:104, 0:256], TANH)
                nc.scalar.activation(gf[:], mx[0:104, 256:512], SIG)
                nc.scalar.activation(go[:], mx[0:104, 512:768], SIG)
                h_t = ssb.tile([104, 256], FP16, tag="h")
                nc.vector.tensor_mul(h_t[:], go[:], c_t[:])
                store_h(h_t, 0)
                hT = transpose_h(h_t)

                for t in range(1, t_steps):
                    mlx_t = mlx_next
                    mlx_next = load_mlx(t + 1) if t + 1 < t_steps else None
                    mx = mx_pass(mlx_t)
                    pg0 = spg.tile([128, 512], FP32, tag="pg0")
                    # separate PSUM banks per half so half-0 EW isn't
                    # bank-serialized behind half-1 matmul writes
                    pgh = [
                        [
                            spg.tile([128, 256], FP32, tag=f"pg{r}h{hf}", name=f"pg{r}h{hf}")
                            for hf in range(2)
                        ]
                        for r in (1, 2)
                    ]
                    # round c: full 512-col k-loop
                    for i, k in enumerate(KORDER):
                        lhs = hT[:, k % 2, 8 * (k // 2) : 8 * (k // 2) + 8]
                        for q in range(4):
                            nc.tensor.matmul(
                                pg0[32 * q : 32 * q + 8, :],
                                lhs,
                                u_sb[:, k, 1536 * q : 1536 * q + 512],
                                start=(i == 0), stop=(i == 7),
                                tile_position=(0, 32 * q),
                            )
                    # rounds f, o: two half-k-loops, each into its own bank
                    for r in (1, 2):
                        for hf in range(2):
                            for i, k in enumerate(KORDER):
                                lhs = hT[:, k % 2, 8 * (k // 2) : 8 * (k // 2) + 8]
                                for q in range(4):
                                    n0 = 1536 * q + 512 * r + 256 * hf
                                    nc.tensor.matmul(
                                        pgh[r - 1][hf][32 * q : 32 * q + 8, :],
                                        lhs,
                                        u_sb[:, k, n0 : n0 + 256],
                                        start=(i == 0), stop=(i == 7),
                                        tile_position=(0, 32 * q),
                                    )
                    # z gate (round c) full width
                    sp = ssb.tile([104, 256], FP32, tag="sp")
                    nc.scalar.activation(sp[:], pg0[0:104, 0:256], SIG)
                    mlh = ssb.tile([104, 256], FP32, tag="mlh")
                    nc.vector.tensor_mul(mlh[:], sp[:], pg0[0:104, 256:512])
                    gin = ssb.tile([104, 256], FP32, tag="gin")
                    nc.vector.tensor_add(gin[:], mlh[:], mx[0:104, 0:256])
                    z_t = ssb.tile([104, 256], FP32, tag="gz")
                    nc.scalar.activation(z_t[:], gin[:], TANH)
                    # f gate + c update, per half
                    c_prev = c_t
                    c_t = ssb.tile([104, 256], FP32, tag="c")
                    for hf in range(2):
                        cs = slice(128 * hf, 128 * hf + 128)
                        spf = ssb.tile([104, 128], FP32, tag="spf")
                        nc.scalar.activation(spf[:], pgh[0][hf][0:104, 0:128], SIG)
                        mlhf = ssb.tile([104, 128], FP32, tag="mlhf")
                        nc.vector.tensor_mul(
                            mlhf[:], spf[:], pgh[0][hf][0:104, 128:256]
                        )
                        ginf = ssb.tile([104, 128], FP32, tag="ginf")
                        nc.vector.tensor_add(
                            ginf[:], mlhf[:], mx[0:104, 256 + 128 * hf : 384 + 128 * hf]
                        )
                        f_h = ssb.tile([104, 128], FP32, tag="gfh")
                        nc.scalar.activation(f_h[:], ginf[:], SIG)
                        fc = ssb.tile([104, 128], FP32, tag="fc")
                        nc.vector.tensor_mul(fc[:], f_h[:], c_prev[:, cs])
                        nc.vector.tensor_add(c_t[:, cs], fc[:], z_t[:, cs])
                    h_t = ssb.tile([104, 256], FP16, tag="h")
                    last = t + 1 >= t_steps
                    pT = None if last else spt.tile([128, 2, 32], FP32, tag="pT")
                    hT_new = None if last else ssb.tile([128, 2, 32], FP16, tag="hT")
                    for hf in range(2):
                        spo = ssb.tile([104, 128], FP32, tag="spo")
                        nc.scalar.activation(spo[:], pgh[1][hf][0:104, 0:128], SIG)
                        mlho = ssb.tile([104, 128], FP32, tag="mlho")
                        nc.vector.tensor_mul(
                            mlho[:], spo[:], pgh[1][hf][0:104, 128:256]
                        )
                        gino = ssb.tile([104, 128], FP32, tag="gino")
                        nc.vector.tensor_add(
                            gino[:], mlho[:], mx[0:104, 512 + 128 * hf : 640 + 128 * hf]
                        )
                        go_h = ssb.tile([104, 128], FP32, tag="goh")
                        nc.scalar.activation(go_h[:], gino[:], SIG)
                        nc.vector.tensor_mul(
                            h_t[:, 128 * hf : 128 * hf + 128],
                            go_h[:],
                            c_t[:, 128 * hf : 128 * hf + 128],
                        )
                        if not last:
                            for cb in range(4):
                                nc.tensor.matmul(
                                    pT[32 * cb : 32 * cb + 32, hf, :],
                                    h_t[:, 128 * hf + 32 * cb : 128 * hf + 32 * cb + 32],
                                    sel[:],
                                    start=True, stop=True,
                                    tile_position=(0, 32 * cb),
                                )
                            nc.scalar.copy(hT_new[:, hf, :], pT[:, hf, :])
                    store_h(h_t, t)
                    hT = hT_new
                nc.sync.dma_start(cT_d[:], c_t[:])
    nc.compile()
    return nc


def _prep_weights(kw):
    """Build host-side constant tensors (layout/cast only)."""
    gorder = ("c", "f", "o")
    w_all = np.zeros((512, NU), np.float32)
    u_all = np.zeros((1024, NU), np.float32)
    bias = np.zeros((NX,), np.float32)
    for g_i, g in enumerate(gorder):
        Wl, Wm = kw[f"W_{g}"], kw[f"W_{g}_mask"]
        Ul, Um = kw[f"U_{g}"], kw[f"U_{g}_mask"]
        for q in range(4):
            n0 = 1536 * q + 512 * g_i
            w_all[:, n0 : n0 + 256] = Wm[256 * q : 256 * q + 256, :].T
            w_all[:, n0 + 256 : n0 + 512] = Wl[256 * q : 256 * q + 256, :].T
            if g_i < 1:
                u_all[:, n0 : n0 + 256] = Um[256 * q : 256 * q + 256, :].T
                u_all[:, n0 + 256 : n0 + 512] = Ul[256 * q : 256 * q + 256, :].T
            else:
                # f/o gates: [pre_h0 lin_h0 pre_h1 lin_h1] interleave (128 each)
                for hh in range(2):
                    base = n0 + 256 * hh
                    r0 = 256 * q + 128 * hh
                    u_all[:, base : base + 128] = Um[r0 : r0 + 128, :].T
                    u_all[:, base + 128 : base + 256] = Ul[r0 : r0 + 128, :].T
            bias[768 * q + 256 * g_i : 768 * q + 256 * g_i + 256] = kw[f"b_{g}"][
                256 * q : 256 * q + 256
            ]
    w4 = w_all.reshape(4, 128, NU).astype(np.float16)
    u8 = u_all.reshape(8, 128, NU).astype(np.float16)
    bias16 = bias.reshape(1, NX).astype(np.float16)
    iaug = np.zeros((9, 8), np.float16)
    iaug[0:8, 0:8] = np.eye(8)
    iaug[8, :] = 1.0
    sel = np.zeros((104, 32), np.float16)
    for q in range(4):
        for b in range(8):
            sel[32 * q + b, 8 * q + b] = 1.0
    return w4, u8, bias16, iaug, sel


def kernel(**inputs):
    x = np.asarray(inputs["input"], np.float32)
    w4, u8, bias16, iaug, sel = _prep_weights(inputs)
    nc = build(T_STEPS)
    in_maps = []
    for c in range(NCORES):
        xc = x[BL * c : BL * c + BL].reshape(ROWS, IN)
        xT = np.ascontiguousarray(xc.T.reshape(4, 128, ROWS)).astype(np.float16)
        in_maps.append(
            dict(xT=xT, w=w4, u=u8, bias=bias16, iaug=iaug, sel=sel)
        )
    res = run_bass_kernel_spmd(nc, in_maps, core_ids=list(range(NCORES)))
    global LAST_RESULT
    LAST_RESULT = res
    out = np.zeros((B, T, H), np.float32)
    cT = np.zeros((B, H), np.float32)
    for c in range(NCORES):
        ys = res.results[c]["ys"].astype(np.float32)
        out[BL * c : BL * c + BL] = ys
        cc = res.results[c]["cT"]
        for q in range(4):
            cT[BL * c : BL * c + BL, 256 * q : 256 * q + 256] = cc[
                32 * q : 32 * q + 8, :
            ]
    hT = out[:, T_STEPS - 1, :].copy()
    return out, (hT[None], cT[None])
